# revision 30
# baseline (speedup 1.0000x reference)
"""Grouped-Query Attention (B=2, S=2048, D=2048, 16 Q heads / 4 KV heads,
hd=128, RoPE, causal) on 8 trn2 NeuronCores.

Sharding: mesh = 2 (batch) x 4 (KV-head groups).  Core c = b*4 + g gets
batch b and KV head g together with its 4 query heads (tensor parallel on
the head dim: q/k/v projection output dim and o-proj input dim).  Each core
produces a partial y[b] (o-proj over its 512 input dims); host sums the 4
partials per batch.

On-chip layout: all activations transposed ([feature, seq]) so every matmul
contracts along the partition dim.  DMA'd tensors (x, weights, y) travel as
bf16 (half the HBM traffic, same 1-cycle/row PE rate); on-chip
intermediates (q/k/v, exp scores) stay float32r.  Softmax is unnormalized:
exp(scale*s) via ACT, denominator via an all-ones [128,128] stationary
matmul that lands pre-broadcast in PSUM, reciprocal on the ACT engine,
causal mask as a post-exp 0/1 multiply on DVE.
"""

import os

import numpy as np

S = 2048
D = 2048
HD = 128
NQH = 16
NKVH = 4
GROUPS = NQH // NKVH  # 4 q heads per kv head
O = GROUPS * HD  # 512 per-core q/o slice
NB = 2
NCORES = 8
SCALE = 1.0 / float(np.sqrt(np.float32(HD)))

SBLK = 512  # seq block for projections / sq block in attention
NKB = S // HD  # 16 128-blocks along seq
NSB = S // SBLK  # 4 512-blocks along seq
NDB = D // HD  # 16 d blocks

LAST_EXEC_NS = None
LAST_TRACE = None

_CACHE = {}


def _rope_tables():
    k = np.arange(0, HD, 2)[: HD // 2].astype(np.float32)
    inv_freq = (1.0 / 10000.0 ** (k / HD)).astype(np.float32)
    positions = np.arange(S, dtype=np.float32)
    ang = positions[:, None] * inv_freq[None, :]  # [S, 64]
    ang = np.concatenate([ang, ang], axis=-1)  # [S, 128]
    cosT = np.cos(ang).astype(np.float32).T  # [128, S]
    sinT = np.sin(ang).astype(np.float32).T
    return np.ascontiguousarray(cosT), np.ascontiguousarray(sinT)


def _mask_table():
    # maskM[i, j*512 + s] = 1 if (j*128 + i) <= s else 0  (keep-mask)
    m = np.empty((HD, 4 * SBLK), dtype=np.float32)
    i = np.arange(HD)[:, None]
    s = np.arange(SBLK)[None, :]
    for j in range(4):
        m[:, j * SBLK : (j + 1) * SBLK] = np.where(j * HD + i <= s, 1.0, 0.0)
    return m


def _shift_table():
    # rot = P @ q  with rot[i] = -q[i+64] (i<64), q[i-64] (i>=64); ship P.T
    P = np.zeros((HD, HD), dtype=np.float32)
    h = HD // 2
    P[np.arange(h), np.arange(h) + h] = -1.0
    P[np.arange(h) + h, np.arange(h)] = 1.0
    return np.ascontiguousarray(P.T)


def _build_program():
    import concourse.bass as bass
    import concourse.mybir as mybir
    from concourse.tile import TileContext

    f32 = mybir.dt.float32
    f32r = mybir.dt.float32r
    bf16 = mybir.dt.bfloat16
    EXP = mybir.ActivationFunctionType.Exp
    LN = mybir.ActivationFunctionType.Ln

    nc = bass.Bass()

    xT = nc.declare_dram_parameter("xT", [D, S], bf16, isOutput=False)
    wqP = nc.declare_dram_parameter("wqP", [128, NDB * O], bf16, isOutput=False)
    wkP = nc.declare_dram_parameter("wkP", [128, NDB * HD], bf16, isOutput=False)
    wvP = nc.declare_dram_parameter("wvP", [128, NDB * HD], bf16, isOutput=False)
    woP = nc.declare_dram_parameter("woP", [128, GROUPS * D], bf16, isOutput=False)
    cosT = nc.declare_dram_parameter("cosT", [HD, S], bf16, isOutput=False)
    sinT = nc.declare_dram_parameter("sinT", [HD, S], bf16, isOutput=False)
    maskM = nc.declare_dram_parameter("maskM", [HD, 4 * SBLK], bf16, isOutput=False)
    shiftPT = nc.declare_dram_parameter("shiftPT", [HD, HD], f32r, isOutput=False)
    ident = nc.declare_dram_parameter("ident", [HD, HD], f32r, isOutput=False)
    onesmat = nc.declare_dram_parameter("onesmat", [HD, HD], f32r, isOutput=False)
    y = nc.declare_dram_parameter("y", [S, D], bf16, isOutput=True)

    with TileContext(nc) as tc:
        with tc.tile_pool(name="persist", bufs=1) as pp:
            wq_sb = pp.tile([128, NDB * O], bf16, name="wq_sb")  # [d_blk][128d, 512o]
            wk_sb = pp.tile([128, NDB * HD], bf16, name="wk_sb")
            wv_sb = pp.tile([128, NDB * HD], bf16, name="wv_sb")
            wo_sb = pp.tile([128, GROUPS * D], bf16, name="wo_sb")  # [o_blk][128o, 2048]
            cos_sb = pp.tile([128, S], bf16, name="cos_sb")
            sin_sb = pp.tile([128, S], bf16, name="sin_sb")
            mask_sb = pp.tile([128, 4 * SBLK], bf16, name="mask_sb")
            shift_sb = pp.tile([128, HD], f32r, name="shift_sb")
            id_sb = pp.tile([128, HD], f32r, name="id_sb")
            ones_sb = pp.tile([128, HD], f32r, name="ones_sb")
            q_sb = pp.tile([128, GROUPS * S], f32r, name="q_sb")  # per head [128hd, S]
            k_sb = pp.tile([128, S], f32r, name="k_sb")
            v_sb = pp.tile([128, NKB * HD], f32r, name="v_sb")  # [s_blk][128s, 128hd]

            # weight loads in 4-db chunks on the scalar (ACT) DMA queue
            # (nothing else rides scalar, so phase-1 ACT copies are never
            # stuck behind DMA issue depth-waits); small tables + sb0's
            # cos/sin chunk lead the sync queue, x tiles follow
            CH = 4
            for c in range(NDB // CH):
                nc.scalar.dma_start(
                    out=wq_sb[:, c * CH * O : (c + 1) * CH * O],
                    in_=wqP[:, c * CH * O : (c + 1) * CH * O],
                )
                nc.scalar.dma_start(
                    out=wk_sb[:, c * CH * HD : (c + 1) * CH * HD],
                    in_=wkP[:, c * CH * HD : (c + 1) * CH * HD],
                )
            nc.sync.dma_start(out=shift_sb[:], in_=shiftPT[:])
            nc.sync.dma_start(out=id_sb[:], in_=ident[:])
            nc.sync.dma_start(out=ones_sb[:], in_=onesmat[:])
            nc.sync.dma_start(out=cos_sb[:, :SBLK], in_=cosT[:, :SBLK])
            nc.sync.dma_start(out=sin_sb[:, :SBLK], in_=sinT[:, :SBLK])

            # ---------------- Phase 1: projections + RoPE + v transpose
            with (
                tc.tile_pool(name="p1acc", bufs=6, space="PSUM") as accp,
                tc.tile_pool(name="p1rot", bufs=2, space="PSUM") as rotp,
                tc.tile_pool(name="xts", bufs=4) as xpool,
                tc.tile_pool(name="raws", bufs=8) as rawpool,
                tc.tile_pool(name="tmps", bufs=4) as tmppool,
            ):
                for sb in range(NSB):
                    sl = slice(sb * SBLK, (sb + 1) * SBLK)
                    ps = [
                        accp.tile([128, SBLK], f32, name=f"acc{i}_{sb}", tag="acc")
                        for i in range(6)
                    ]  # q0..q3, k, v
                    for db in range(NDB):
                        if sb == 0 and db % CH == 0:
                            c = db // CH
                            nc.sync.dma_start(
                                out=wv_sb[:, c * CH * HD : (c + 1) * CH * HD],
                                in_=wvP[:, c * CH * HD : (c + 1) * CH * HD],
                            )
                        xt = xpool.tile([128, SBLK], bf16, name=f"xt{sb}_{db}", tag="xt")
                        nc.sync.dma_start(
                            out=xt[:], in_=xT[db * 128 : (db + 1) * 128, sl]
                        )
                        st = db == 0
                        sp = db == NDB - 1
                        for ob in range(GROUPS):
                            nc.tensor.matmul(
                                ps[ob][:],
                                wq_sb[:, db * O + ob * 128 : db * O + (ob + 1) * 128],
                                xt[:],
                                start=st,
                                stop=sp,
                            )
                        nc.tensor.matmul(
                            ps[4][:],
                            wk_sb[:, db * HD : (db + 1) * HD],
                            xt[:],
                            start=st,
                            stop=sp,
                        )
                        nc.tensor.matmul(
                            ps[5][:],
                            wv_sb[:, db * HD : (db + 1) * HD],
                            xt[:],
                            start=st,
                            stop=sp,
                        )
                    if sb < NSB - 1:
                        nsl = slice((sb + 1) * SBLK, (sb + 2) * SBLK)
                        nc.sync.dma_start(out=cos_sb[:, nsl], in_=cosT[:, nsl])
                        nc.sync.dma_start(out=sin_sb[:, nsl], in_=sinT[:, nsl])
                    # stage all six psum accumulators to SBUF first: the
                    # accp banks free after ~4us of ACT copies, so the next
                    # sb's projections never wait on the rope/transpose chain
                    raws = []
                    for i in range(5):
                        raw = rawpool.tile([128, SBLK], f32r, name=f"raw{sb}_{i}", tag="raw")
                        nc.scalar.copy(raw[:], ps[i][:])
                        raws.append(raw)
                    vst = rawpool.tile([128, SBLK], f32r, name=f"vst{sb}", tag="raw")
                    nc.scalar.copy(vst[:], ps[5][:])
                    # RoPE on q heads and k
                    for i in range(5):
                        dst = (
                            q_sb[:, i * S + sb * SBLK : i * S + (sb + 1) * SBLK]
                            if i < 4
                            else k_sb[:, sl]
                        )
                        raw = raws[i]
                        rot = rotp.tile([128, SBLK], f32, name=f"rot{sb}_{i}", tag="rot")
                        nc.tensor.matmul(
                            rot[:], shift_sb[:], raw[:], start=True, stop=True
                        )
                        tmp = tmppool.tile([128, SBLK], f32, name=f"tmp{sb}_{i}", tag="tmp")
                        nc.vector.tensor_mul(tmp[:], raw[:], cos_sb[:, sl])
                        t2 = tmppool.tile([128, SBLK], f32, name=f"t2_{sb}_{i}", tag="tmp")
                        nc.vector.tensor_mul(t2[:], rot[:], sin_sb[:, sl])
                        nc.vector.tensor_add(dst, tmp[:], t2[:])
                    # v: PE-transpose 128x128 blocks to natural layout
                    for sub in range(SBLK // HD):
                        vt = rotp.tile([128, SBLK], f32r, name=f"vt{sb}_{sub}", tag="rot")
                        nc.tensor.transpose(
                            vt[:, :HD], vst[:, sub * HD : (sub + 1) * HD], id_sb[:]
                        )
                        kb = sb * 4 + sub
                        nc.scalar.copy(v_sb[:, kb * HD : (kb + 1) * HD], vt[:, :HD])

            nc.sync.dma_start(out=mask_sb[:], in_=maskM[:])
            nc.sync.dma_start(out=wo_sb[:], in_=woP[:])

            # ---------------- Phase 2: attention + o-proj, per sq block.
            # kb blocks processed in pairs: sc/e tiles are [128, 1024] so
            # the ACT exp and DVE mask amortize their fixed access latency.
            with (
                tc.tile_pool(name="p2sc", bufs=2, space="PSUM") as scp,
                tc.tile_pool(name="p2av", bufs=2, space="PSUM") as avp,
                tc.tile_pool(name="p2den", bufs=2, space="PSUM") as denp,
                tc.tile_pool(name="exps", bufs=6) as epool,
                tc.tile_pool(name="recs", bufs=4) as recpool,
                tc.tile_pool(name="aos", bufs=8) as aopool,
                tc.tile_pool(name="ysb", bufs=3) as ypool_sb,
            ):
                pending_oproj = []

                for sq in range(NSB):
                    nsk = 4 * sq + 4
                    aoh = [
                        aopool.tile([128, SBLK], bf16, name=f"ao{sq}_{h}", tag="ao")
                        for h in range(GROUPS)
                    ]
                    # flattened (head, pair) stream with 1-pair sc prefetch:
                    # the sc matmuls of pair i+1 are issued while pair i's
                    # exp is still on the ACT engine, so the PE never waits
                    # a full exp latency at head boundaries.  Diagonal
                    # (masked) pairs go first within each head so the DVE
                    # mask-mul hides behind the long unmasked run.
                    kps = [2 * sq, 2 * sq + 1] + list(range(2 * sq))
                    stream = [(h, ki, kp) for h in range(GROUPS) for ki, kp in enumerate(kps)]
                    np2 = len(kps)

                    sc_t = {}
                    avd = {}

                    def emit_sc(idx):
                        h, ki, kp = stream[idx]
                        if ki == 0:
                            avd[h] = (
                                avp.tile([128, SBLK], f32, name=f"av{sq}_{h}", tag="av"),
                                denp.tile([128, SBLK], f32, name=f"den{sq}_{h}", tag="den"),
                            )
                        qsl = q_sb[:, h * S + sq * SBLK : h * S + (sq + 1) * SBLK]
                        sc = scp.tile([128, 2 * SBLK], f32, name=f"sc{sq}_{h}_{kp}", tag="sc")
                        for half, kb in ((0, 2 * kp), (1, 2 * kp + 1)):
                            nc.tensor.matmul(
                                sc[:, half * SBLK : (half + 1) * SBLK],
                                k_sb[:, kb * HD : (kb + 1) * HD],
                                qsl,
                                start=True,
                                stop=True,
                            )
                        e = epool.tile([128, 2 * SBLK], f32r, name=f"e{sq}_{h}_{kp}", tag="e")
                        if kp >= 2 * sq:
                            # diagonal pair: exp+mask per 512-half so the
                            # first av matmul waits half the chain latency
                            j2 = kp - 2 * sq
                            for hf in range(2):
                                hsl = slice(hf * SBLK, (hf + 1) * SBLK)
                                nc.scalar.activation(e[:, hsl], sc[:, hsl], EXP, scale=SCALE)
                                nc.vector.tensor_mul(
                                    e[:, hsl],
                                    e[:, hsl],
                                    mask_sb[:, j2 * 2 * SBLK + hf * SBLK : j2 * 2 * SBLK + (hf + 1) * SBLK],
                                )
                        else:
                            nc.scalar.activation(e[:], sc[:], EXP, scale=SCALE)
                        sc_t[idx] = e

                    emit_sc(0)
                    for idx in range(len(stream)):
                        h, ki, kp = stream[idx]
                        if idx + 1 < len(stream):
                            emit_sc(idx + 1)
                        if idx in (2, 6) and pending_oproj:
                            # previous sq's o-proj in two 4-block chunks:
                            # exp-independent PE work that lets the ACT
                            # engine catch up on exps mid-stream
                            for _ in range(min(4, len(pending_oproj))):
                                pending_oproj.pop(0)()
                        e = sc_t.pop(idx)
                        av, den = avd[h]
                        for half, kb in ((0, 2 * kp), (1, 2 * kp + 1)):
                            esl = e[:, half * SBLK : (half + 1) * SBLK]
                            st = ki == 0 and half == 0
                            sp = ki == np2 - 1 and half == 1
                            nc.tensor.matmul(
                                av[:],
                                v_sb[:, kb * HD : (kb + 1) * HD],
                                esl,
                                start=st,
                                stop=sp,
                            )
                            nc.tensor.matmul(den[:], ones_sb[:], esl, start=st, stop=sp)
                        if ki == np2 - 1:
                            # 1/den as exp(-ln(den)) on the ACT engine (keeps
                            # the DVE free; ACT Reciprocal is gated in bass)
                            lnt = recpool.tile([128, SBLK], f32, name=f"ln{sq}_{h}", tag="rec")
                            nc.scalar.activation(lnt[:], den[:], LN)
                            rec = recpool.tile([128, SBLK], f32, name=f"rec{sq}_{h}", tag="rec")
                            nc.scalar.activation(rec[:], lnt[:], EXP, scale=-1.0)
                            nc.vector.tensor_mul(aoh[h][:], av[:], rec[:])
                    # o-proj for this sq block, deferred into the next
                    # sq's pair stream (final sq drains at the very end,
                    # alternating ysb copies across ACT/DVE)
                    def _mk_oproj_block(sq, aoh, sub, dcp):
                        def _go(final=False):
                            yt = scp.tile(
                                [128, 2 * SBLK], f32, name=f"y{sq}_{sub}_{dcp}", tag="sc"
                            )
                            for half in range(2):
                                dc = 2 * dcp + half
                                for ob in range(GROUPS):
                                    nc.tensor.matmul(
                                        yt[:, half * SBLK : (half + 1) * SBLK],
                                        aoh[ob][:, sub * HD : (sub + 1) * HD],
                                        wo_sb[:, ob * D + dc * SBLK : ob * D + (dc + 1) * SBLK],
                                        start=(ob == 0),
                                        stop=(ob == GROUPS - 1),
                                    )
                            ysb = ypool_sb.tile(
                                [128, 2 * SBLK], bf16, name=f"ysb{sq}_{sub}_{dcp}", tag="ysb"
                            )
                            if final and (sub + dcp) % 2 == 0:
                                nc.scalar.copy(ysb[:], yt[:])
                            else:
                                nc.vector.tensor_copy(ysb[:], yt[:])
                            nc.sync.dma_start(
                                out=y[
                                    sq * SBLK + sub * HD : sq * SBLK + (sub + 1) * HD,
                                    dcp * 2 * SBLK : (dcp + 1) * 2 * SBLK,
                                ],
                                in_=ysb[:],
                            )
                        return _go
                    for sub in range(SBLK // HD):
                        for dcp in range(D // (2 * SBLK)):
                            pending_oproj.append(_mk_oproj_block(sq, aoh, sub, dcp))
                for blk in pending_oproj:
                    blk(final=True)
    _split_matmul_waits(nc, mybir)
    return nc


def _split_matmul_waits(nc, mybir):
    """TRN2 instructions can carry only one HW sync-wait command; Tile
    sometimes attaches several.  Move the extras onto nofuse nops on the
    same engine inserted just before the instruction."""
    for f in nc.m.functions:
        for bb in f.blocks:
            insts = bb.instructions
            fixes = []
            for idx, inst in enumerate(insts):
                si = inst.sync_info
                if si is None or len(si.on_wait) <= 1:
                    continue
                fixes.append((idx, inst, list(si.on_wait), list(si.on_update)))
            for idx, inst, waits, updates in reversed(fixes):
                inst.sync_info = mybir.SyncInfo(on_wait=[waits[-1]], on_update=updates)
                for w in reversed(waits[:-1]):
                    nop = mybir.InstNoOp(
                        name=nc.get_next_instruction_name(), ins=[], outs=[]
                    )
                    nop.engine = inst.engine
                    nop.bass_nofuse = True
                    nop.sync_info = mybir.SyncInfo(on_wait=[w], on_update=[])
                    insts.insert(idx, nop)


def _per_core_inputs(x, Wq, Wk, Wv, Wo):
    import ml_dtypes

    bf16 = ml_dtypes.bfloat16
    cosT, sinT = _rope_tables()
    maskM = _mask_table()
    shiftPT = _shift_table()
    ident = np.eye(HD, dtype=np.float32)
    onesmat = np.ones((HD, HD), dtype=np.float32)
    in_maps = []
    for b in range(NB):
        xTb = np.ascontiguousarray(x[b].T.astype(bf16))
        for g in range(NKVH):
            wqT = Wq[g * O : (g + 1) * O, :].T  # [D, O]
            wkT = Wk[g * HD : (g + 1) * HD, :].T
            wvT = Wv[g * HD : (g + 1) * HD, :].T
            woT = Wo[:, g * O : (g + 1) * O].T  # [O, D]
            in_maps.append(
                {
                    "xT": xTb,
                    "wqP": np.ascontiguousarray(
                        wqT.reshape(NDB, 128, O).transpose(1, 0, 2).reshape(128, NDB * O)
                    ).astype(bf16),
                    "wkP": np.ascontiguousarray(
                        wkT.reshape(NDB, 128, HD).transpose(1, 0, 2).reshape(128, NDB * HD)
                    ).astype(bf16),
                    "wvP": np.ascontiguousarray(
                        wvT.reshape(NDB, 128, HD).transpose(1, 0, 2).reshape(128, NDB * HD)
                    ).astype(bf16),
                    "woP": np.ascontiguousarray(
                        woT.reshape(GROUPS, 128, D).transpose(1, 0, 2).reshape(128, GROUPS * D)
                    ).astype(bf16),
                    "cosT": cosT.astype(bf16),
                    "sinT": sinT.astype(bf16),
                    "maskM": maskM.astype(bf16),
                    "shiftPT": shiftPT,
                    "ident": ident,
                    "onesmat": onesmat,
                }
            )
    return in_maps


def kernel(x, Wq, Wk, Wv, Wo):
    global LAST_EXEC_NS, LAST_TRACE
    from concourse.bass_utils import run_bass_kernel_spmd

    if "nc" not in _CACHE:
        _CACHE["nc"] = _build_program()
    nc = _CACHE["nc"]

    x = np.asarray(x)
    in_maps = _per_core_inputs(
        x, np.asarray(Wq), np.asarray(Wk), np.asarray(Wv), np.asarray(Wo)
    )
    trace = bool(os.environ.get("KERNEL_PROFILE"))
    res = run_bass_kernel_spmd(
        nc, in_maps, core_ids=list(range(NCORES)), trace=trace
    )
    globals()["LAST_RESULT"] = res
    LAST_EXEC_NS = res.exec_time_ns
    LAST_TRACE = getattr(res, "profile_json", None)
    out = np.empty((NB, S, D), dtype=np.float32)
    for b in range(NB):
        acc = res.results[b * NKVH]["y"].astype(np.float32)
        for g in range(1, NKVH):
            acc += res.results[b * NKVH + g]["y"].astype(np.float32)
        out[b] = acc
    return out


# revision 31
# speedup vs baseline: 1.0492x; 1.0492x over previous
"""Grouped-Query Attention (B=2, S=2048, D=2048, 16 Q heads / 4 KV heads,
hd=128, RoPE, causal) on 8 trn2 NeuronCores.

Sharding: mesh = 2 (batch) x 4 (KV-head groups).  Core c = b*4 + g gets
batch b and KV head g together with its 4 query heads (tensor parallel on
the head dim: q/k/v projection output dim and o-proj input dim).  Each core
produces a partial y[b] (o-proj over its 512 input dims); host sums the 4
partials per batch.

On-chip layout: all activations transposed ([feature, seq]) so every matmul
contracts along the partition dim.  DMA'd tensors (x, weights, y) travel as
bf16 (half the HBM traffic, same 1-cycle/row PE rate); on-chip
intermediates (q/k/v, exp scores) stay float32r.  Softmax is unnormalized:
exp(scale*s) via ACT, denominator via an all-ones [128,128] stationary
matmul that lands pre-broadcast in PSUM, reciprocal on the ACT engine,
causal mask as a post-exp 0/1 multiply on DVE.
"""

import os

import numpy as np

S = 2048
D = 2048
HD = 128
NQH = 16
NKVH = 4
GROUPS = NQH // NKVH  # 4 q heads per kv head
O = GROUPS * HD  # 512 per-core q/o slice
NB = 2
NCORES = 8
SCALE = 1.0 / float(np.sqrt(np.float32(HD)))

SBLK = 512  # seq block for projections / sq block in attention
NKB = S // HD  # 16 128-blocks along seq
NSB = S // SBLK  # 4 512-blocks along seq
NDB = D // HD  # 16 d blocks

LAST_EXEC_NS = None
LAST_TRACE = None

_CACHE = {}


def _rope_tables():
    k = np.arange(0, HD, 2)[: HD // 2].astype(np.float32)
    inv_freq = (1.0 / 10000.0 ** (k / HD)).astype(np.float32)
    positions = np.arange(S, dtype=np.float32)
    ang = positions[:, None] * inv_freq[None, :]  # [S, 64]
    ang = np.concatenate([ang, ang], axis=-1)  # [S, 128]
    cosT = np.cos(ang).astype(np.float32).T  # [128, S]
    sinT = np.sin(ang).astype(np.float32).T
    return np.ascontiguousarray(cosT), np.ascontiguousarray(sinT)


def _mask_table():
    # maskM[i, j*512 + s] = 1 if (j*128 + i) <= s else 0  (keep-mask)
    m = np.empty((HD, 4 * SBLK), dtype=np.float32)
    i = np.arange(HD)[:, None]
    s = np.arange(SBLK)[None, :]
    for j in range(4):
        m[:, j * SBLK : (j + 1) * SBLK] = np.where(j * HD + i <= s, 1.0, 0.0)
    return m


def _shift_table():
    # rot = P @ q  with rot[i] = -q[i+64] (i<64), q[i-64] (i>=64); ship P.T
    P = np.zeros((HD, HD), dtype=np.float32)
    h = HD // 2
    P[np.arange(h), np.arange(h) + h] = -1.0
    P[np.arange(h) + h, np.arange(h)] = 1.0
    return np.ascontiguousarray(P.T)


def _build_program():
    import concourse.bass as bass
    import concourse.mybir as mybir
    from concourse.tile import TileContext

    f32 = mybir.dt.float32
    f32r = mybir.dt.float32r
    bf16 = mybir.dt.bfloat16
    EXP = mybir.ActivationFunctionType.Exp
    LN = mybir.ActivationFunctionType.Ln

    nc = bass.Bass()

    xT = nc.declare_dram_parameter("xT", [D, S], bf16, isOutput=False)
    wqP = nc.declare_dram_parameter("wqP", [128, NDB * O], bf16, isOutput=False)
    wkP = nc.declare_dram_parameter("wkP", [128, NDB * HD], bf16, isOutput=False)
    wvP = nc.declare_dram_parameter("wvP", [128, NDB * HD], bf16, isOutput=False)
    woP = nc.declare_dram_parameter("woP", [128, GROUPS * D], bf16, isOutput=False)
    cosT = nc.declare_dram_parameter("cosT", [HD, S], bf16, isOutput=False)
    sinT = nc.declare_dram_parameter("sinT", [HD, S], bf16, isOutput=False)
    maskM = nc.declare_dram_parameter("maskM", [HD, 4 * SBLK], bf16, isOutput=False)
    shiftPT = nc.declare_dram_parameter("shiftPT", [HD, HD], f32r, isOutput=False)
    ident = nc.declare_dram_parameter("ident", [HD, HD], f32r, isOutput=False)
    onesmat = nc.declare_dram_parameter("onesmat", [HD, HD], f32r, isOutput=False)
    y = nc.declare_dram_parameter("y", [S, D], bf16, isOutput=True)

    with TileContext(nc) as tc:
        with tc.tile_pool(name="persist", bufs=1) as pp:
            wq_sb = pp.tile([128, NDB * O], bf16, name="wq_sb")  # [d_blk][128d, 512o]
            wk_sb = pp.tile([128, NDB * HD], bf16, name="wk_sb")
            wv_sb = pp.tile([128, NDB * HD], bf16, name="wv_sb")
            wo_sb = pp.tile([128, GROUPS * D], bf16, name="wo_sb")  # [o_blk][128o, 2048]
            cos_sb = pp.tile([128, S], bf16, name="cos_sb")
            sin_sb = pp.tile([128, S], bf16, name="sin_sb")
            mask_sb = pp.tile([128, 4 * SBLK], bf16, name="mask_sb")
            shift_sb = pp.tile([128, HD], f32r, name="shift_sb")
            id_sb = pp.tile([128, HD], f32r, name="id_sb")
            ones_sb = pp.tile([128, HD], f32r, name="ones_sb")
            q_sb = pp.tile([128, GROUPS * S], f32r, name="q_sb")  # per head [128hd, S]
            k_sb = pp.tile([128, S], f32r, name="k_sb")
            v_sb = pp.tile([128, NKB * HD], f32r, name="v_sb")  # [s_blk][128s, 128hd]

            # weight loads in 4-db chunks on the scalar (ACT) DMA queue
            # (nothing else rides scalar, so phase-1 ACT copies are never
            # stuck behind DMA issue depth-waits); small tables + sb0's
            # cos/sin chunk lead the sync queue, x tiles follow
            CH = 4
            for c in range(NDB // CH):
                nc.scalar.dma_start(
                    out=wq_sb[:, c * CH * O : (c + 1) * CH * O],
                    in_=wqP[:, c * CH * O : (c + 1) * CH * O],
                )
                nc.scalar.dma_start(
                    out=wk_sb[:, c * CH * HD : (c + 1) * CH * HD],
                    in_=wkP[:, c * CH * HD : (c + 1) * CH * HD],
                )
            nc.sync.dma_start(out=shift_sb[:], in_=shiftPT[:])
            nc.sync.dma_start(out=id_sb[:], in_=ident[:])
            nc.sync.dma_start(out=ones_sb[:], in_=onesmat[:])
            nc.sync.dma_start(out=cos_sb[:, :SBLK], in_=cosT[:, :SBLK])
            nc.sync.dma_start(out=sin_sb[:, :SBLK], in_=sinT[:, :SBLK])

            # ---------------- Phase 1: projections + RoPE + v transpose
            with (
                tc.tile_pool(name="p1acc", bufs=6, space="PSUM") as accp,
                tc.tile_pool(name="p1rot", bufs=2, space="PSUM") as rotp,
                tc.tile_pool(name="xts", bufs=4) as xpool,
                tc.tile_pool(name="raws", bufs=8) as rawpool,
                tc.tile_pool(name="tmps", bufs=4) as tmppool,
            ):
                for sb in range(NSB):
                    sl = slice(sb * SBLK, (sb + 1) * SBLK)
                    ps = [
                        accp.tile([128, SBLK], f32, name=f"acc{i}_{sb}", tag="acc")
                        for i in range(6)
                    ]  # q0..q3, k, v
                    for db in range(NDB):
                        if sb == 0 and db % CH == 0:
                            c = db // CH
                            nc.sync.dma_start(
                                out=wv_sb[:, c * CH * HD : (c + 1) * CH * HD],
                                in_=wvP[:, c * CH * HD : (c + 1) * CH * HD],
                            )
                        xt = xpool.tile([128, SBLK], bf16, name=f"xt{sb}_{db}", tag="xt")
                        nc.sync.dma_start(
                            out=xt[:], in_=xT[db * 128 : (db + 1) * 128, sl]
                        )
                        st = db == 0
                        sp = db == NDB - 1
                        for ob in range(GROUPS):
                            nc.tensor.matmul(
                                ps[ob][:],
                                wq_sb[:, db * O + ob * 128 : db * O + (ob + 1) * 128],
                                xt[:],
                                start=st,
                                stop=sp,
                            )
                        nc.tensor.matmul(
                            ps[4][:],
                            wk_sb[:, db * HD : (db + 1) * HD],
                            xt[:],
                            start=st,
                            stop=sp,
                        )
                        nc.tensor.matmul(
                            ps[5][:],
                            wv_sb[:, db * HD : (db + 1) * HD],
                            xt[:],
                            start=st,
                            stop=sp,
                        )
                    if sb < NSB - 1:
                        nsl = slice((sb + 1) * SBLK, (sb + 2) * SBLK)
                        nc.sync.dma_start(out=cos_sb[:, nsl], in_=cosT[:, nsl])
                        nc.sync.dma_start(out=sin_sb[:, nsl], in_=sinT[:, nsl])
                    # stage all six psum accumulators to SBUF first: the
                    # accp banks free after ~4us of ACT copies, so the next
                    # sb's projections never wait on the rope/transpose chain
                    raws = []
                    for i in range(5):
                        raw = rawpool.tile([128, SBLK], f32r, name=f"raw{sb}_{i}", tag="raw")
                        nc.scalar.copy(raw[:], ps[i][:])
                        raws.append(raw)
                    vst = rawpool.tile([128, SBLK], f32r, name=f"vst{sb}", tag="raw")
                    nc.scalar.copy(vst[:], ps[5][:])
                    # RoPE on q heads and k
                    for i in range(5):
                        dst = (
                            q_sb[:, i * S + sb * SBLK : i * S + (sb + 1) * SBLK]
                            if i < 4
                            else k_sb[:, sl]
                        )
                        raw = raws[i]
                        rot = rotp.tile([128, SBLK], f32, name=f"rot{sb}_{i}", tag="rot")
                        nc.tensor.matmul(
                            rot[:], shift_sb[:], raw[:], start=True, stop=True
                        )
                        tmp = tmppool.tile([128, SBLK], f32, name=f"tmp{sb}_{i}", tag="tmp")
                        nc.vector.tensor_mul(tmp[:], raw[:], cos_sb[:, sl])
                        t2 = tmppool.tile([128, SBLK], f32, name=f"t2_{sb}_{i}", tag="tmp")
                        nc.vector.tensor_mul(t2[:], rot[:], sin_sb[:, sl])
                        nc.vector.tensor_add(dst, tmp[:], t2[:])
                    # v: PE-transpose 128x128 blocks to natural layout
                    for sub in range(SBLK // HD):
                        vt = rotp.tile([128, SBLK], f32r, name=f"vt{sb}_{sub}", tag="rot")
                        nc.tensor.transpose(
                            vt[:, :HD], vst[:, sub * HD : (sub + 1) * HD], id_sb[:]
                        )
                        kb = sb * 4 + sub
                        nc.scalar.copy(v_sb[:, kb * HD : (kb + 1) * HD], vt[:, :HD])

            nc.sync.dma_start(out=mask_sb[:], in_=maskM[:])
            nc.sync.dma_start(out=wo_sb[:], in_=woP[:])

            # ---------------- Phase 2: attention + o-proj, per sq block.
            # kb blocks processed in pairs: sc/e tiles are [128, 1024] so
            # the ACT exp and DVE mask amortize their fixed access latency.
            with (
                tc.tile_pool(name="p2sc", bufs=2, space="PSUM") as scp,
                tc.tile_pool(name="p2av", bufs=2, space="PSUM") as avp,
                tc.tile_pool(name="p2den", bufs=2, space="PSUM") as denp,
                tc.tile_pool(name="exps", bufs=6) as epool,
                tc.tile_pool(name="recs", bufs=4) as recpool,
                tc.tile_pool(name="aos", bufs=8) as aopool,
                tc.tile_pool(name="ysb", bufs=3) as ypool_sb,
            ):
                pending_oproj = [None]

                for sq in range(NSB):
                    nsk = 4 * sq + 4
                    aoh = [
                        aopool.tile([128, SBLK], bf16, name=f"ao{sq}_{h}", tag="ao")
                        for h in range(GROUPS)
                    ]
                    # flattened (head, pair) stream with 1-pair sc prefetch:
                    # the sc matmuls of pair i+1 are issued while pair i's
                    # exp is still on the ACT engine, so the PE never waits
                    # a full exp latency at head boundaries.  Diagonal
                    # (masked) pairs go first within each head so the DVE
                    # mask-mul hides behind the long unmasked run.
                    kps = [2 * sq, 2 * sq + 1] + list(range(2 * sq))
                    stream = [(h, ki, kp) for h in range(GROUPS) for ki, kp in enumerate(kps)]
                    np2 = len(kps)

                    sc_t = {}
                    avd = {}

                    def emit_sc(idx):
                        h, ki, kp = stream[idx]
                        if ki == 0:
                            avd[h] = (
                                avp.tile([128, SBLK], f32, name=f"av{sq}_{h}", tag="av"),
                                denp.tile([128, SBLK], f32, name=f"den{sq}_{h}", tag="den"),
                            )
                        qsl = q_sb[:, h * S + sq * SBLK : h * S + (sq + 1) * SBLK]
                        sc = scp.tile([128, 2 * SBLK], f32, name=f"sc{sq}_{h}_{kp}", tag="sc")
                        for half, kb in ((0, 2 * kp), (1, 2 * kp + 1)):
                            nc.tensor.matmul(
                                sc[:, half * SBLK : (half + 1) * SBLK],
                                k_sb[:, kb * HD : (kb + 1) * HD],
                                qsl,
                                start=True,
                                stop=True,
                            )
                        e = epool.tile([128, 2 * SBLK], f32r, name=f"e{sq}_{h}_{kp}", tag="e")
                        nc.scalar.activation(e[:], sc[:], EXP, scale=SCALE)
                        if kp >= 2 * sq:
                            j2 = kp - 2 * sq
                            nc.vector.tensor_mul(
                                e[:], e[:], mask_sb[:, j2 * 2 * SBLK : (j2 + 1) * 2 * SBLK]
                            )
                        sc_t[idx] = e

                    emit_sc(0)
                    for idx in range(len(stream)):
                        h, ki, kp = stream[idx]
                        if idx + 1 < len(stream):
                            emit_sc(idx + 1)
                        if idx == 2 and pending_oproj[0] is not None:
                            # previous sq's o-proj runs here: its last-head
                            # reciprocal chain hides behind this sq's first
                            # score pairs
                            pending_oproj[0]()
                            pending_oproj[0] = None
                        e = sc_t.pop(idx)
                        av, den = avd[h]
                        for half, kb in ((0, 2 * kp), (1, 2 * kp + 1)):
                            esl = e[:, half * SBLK : (half + 1) * SBLK]
                            st = ki == 0 and half == 0
                            sp = ki == np2 - 1 and half == 1
                            nc.tensor.matmul(
                                av[:],
                                v_sb[:, kb * HD : (kb + 1) * HD],
                                esl,
                                start=st,
                                stop=sp,
                            )
                            nc.tensor.matmul(den[:], ones_sb[:], esl, start=st, stop=sp)
                        if ki == np2 - 1:
                            # 1/den as exp(-ln(den)) on the ACT engine (keeps
                            # the DVE free; ACT Reciprocal is gated in bass)
                            lnt = recpool.tile([128, SBLK], f32, name=f"ln{sq}_{h}", tag="rec")
                            nc.scalar.activation(lnt[:], den[:], LN)
                            rec = recpool.tile([128, SBLK], f32, name=f"rec{sq}_{h}", tag="rec")
                            nc.scalar.activation(rec[:], lnt[:], EXP, scale=-1.0)
                            nc.vector.tensor_mul(aoh[h][:], av[:], rec[:])
                    # o-proj for this sq block, deferred into the next
                    # sq's pair stream (final sq drains at the very end,
                    # alternating ysb copies across ACT/DVE)
                    def _mk_oproj(sq, aoh):
                        def _go(final=False):
                            for sub in range(SBLK // HD):
                                for dcp in range(D // (2 * SBLK)):
                                    yt = scp.tile(
                                        [128, 2 * SBLK], f32, name=f"y{sq}_{sub}_{dcp}", tag="sc"
                                    )
                                    for half in range(2):
                                        dc = 2 * dcp + half
                                        for ob in range(GROUPS):
                                            nc.tensor.matmul(
                                                yt[:, half * SBLK : (half + 1) * SBLK],
                                                aoh[ob][:, sub * HD : (sub + 1) * HD],
                                                wo_sb[:, ob * D + dc * SBLK : ob * D + (dc + 1) * SBLK],
                                                start=(ob == 0),
                                                stop=(ob == GROUPS - 1),
                                            )
                                    ysb = ypool_sb.tile(
                                        [128, 2 * SBLK], bf16, name=f"ysb{sq}_{sub}_{dcp}", tag="ysb"
                                    )
                                    if final and (sub + dcp) % 2 == 0:
                                        nc.scalar.copy(ysb[:], yt[:])
                                    else:
                                        nc.vector.tensor_copy(ysb[:], yt[:])
                                    nc.sync.dma_start(
                                        out=y[
                                            sq * SBLK + sub * HD : sq * SBLK + (sub + 1) * HD,
                                            dcp * 2 * SBLK : (dcp + 1) * 2 * SBLK,
                                        ],
                                        in_=ysb[:],
                                    )
                        return _go
                    pending_oproj[0] = _mk_oproj(sq, aoh)
                if pending_oproj[0] is not None:
                    pending_oproj[0](final=True)
    _split_matmul_waits(nc, mybir)
    return nc


def _split_matmul_waits(nc, mybir):
    """TRN2 instructions can carry only one HW sync-wait command; Tile
    sometimes attaches several.  Move the extras onto nofuse nops on the
    same engine inserted just before the instruction."""
    for f in nc.m.functions:
        for bb in f.blocks:
            insts = bb.instructions
            fixes = []
            for idx, inst in enumerate(insts):
                si = inst.sync_info
                if si is None or len(si.on_wait) <= 1:
                    continue
                fixes.append((idx, inst, list(si.on_wait), list(si.on_update)))
            for idx, inst, waits, updates in reversed(fixes):
                inst.sync_info = mybir.SyncInfo(on_wait=[waits[-1]], on_update=updates)
                for w in reversed(waits[:-1]):
                    nop = mybir.InstNoOp(
                        name=nc.get_next_instruction_name(), ins=[], outs=[]
                    )
                    nop.engine = inst.engine
                    nop.bass_nofuse = True
                    nop.sync_info = mybir.SyncInfo(on_wait=[w], on_update=[])
                    insts.insert(idx, nop)


def _per_core_inputs(x, Wq, Wk, Wv, Wo):
    import ml_dtypes

    bf16 = ml_dtypes.bfloat16
    cosT, sinT = _rope_tables()
    maskM = _mask_table()
    shiftPT = _shift_table()
    ident = np.eye(HD, dtype=np.float32)
    onesmat = np.ones((HD, HD), dtype=np.float32)
    in_maps = []
    for b in range(NB):
        xTb = np.ascontiguousarray(x[b].T.astype(bf16))
        for g in range(NKVH):
            wqT = Wq[g * O : (g + 1) * O, :].T  # [D, O]
            wkT = Wk[g * HD : (g + 1) * HD, :].T
            wvT = Wv[g * HD : (g + 1) * HD, :].T
            woT = Wo[:, g * O : (g + 1) * O].T  # [O, D]
            in_maps.append(
                {
                    "xT": xTb,
                    "wqP": np.ascontiguousarray(
                        wqT.reshape(NDB, 128, O).transpose(1, 0, 2).reshape(128, NDB * O)
                    ).astype(bf16),
                    "wkP": np.ascontiguousarray(
                        wkT.reshape(NDB, 128, HD).transpose(1, 0, 2).reshape(128, NDB * HD)
                    ).astype(bf16),
                    "wvP": np.ascontiguousarray(
                        wvT.reshape(NDB, 128, HD).transpose(1, 0, 2).reshape(128, NDB * HD)
                    ).astype(bf16),
                    "woP": np.ascontiguousarray(
                        woT.reshape(GROUPS, 128, D).transpose(1, 0, 2).reshape(128, GROUPS * D)
                    ).astype(bf16),
                    "cosT": cosT.astype(bf16),
                    "sinT": sinT.astype(bf16),
                    "maskM": maskM.astype(bf16),
                    "shiftPT": shiftPT,
                    "ident": ident,
                    "onesmat": onesmat,
                }
            )
    return in_maps


def kernel(x, Wq, Wk, Wv, Wo):
    global LAST_EXEC_NS, LAST_TRACE
    from concourse.bass_utils import run_bass_kernel_spmd

    if "nc" not in _CACHE:
        _CACHE["nc"] = _build_program()
    nc = _CACHE["nc"]

    x = np.asarray(x)
    in_maps = _per_core_inputs(
        x, np.asarray(Wq), np.asarray(Wk), np.asarray(Wv), np.asarray(Wo)
    )
    trace = bool(os.environ.get("KERNEL_PROFILE"))
    res = run_bass_kernel_spmd(
        nc, in_maps, core_ids=list(range(NCORES)), trace=trace
    )
    globals()["LAST_RESULT"] = res
    LAST_EXEC_NS = res.exec_time_ns
    LAST_TRACE = getattr(res, "profile_json", None)
    out = np.empty((NB, S, D), dtype=np.float32)
    for b in range(NB):
        acc = res.results[b * NKVH]["y"].astype(np.float32)
        for g in range(1, NKVH):
            acc += res.results[b * NKVH + g]["y"].astype(np.float32)
        out[b] = acc
    return out


# revision 32
# speedup vs baseline: 1.0499x; 1.0007x over previous
"""Grouped-Query Attention (B=2, S=2048, D=2048, 16 Q heads / 4 KV heads,
hd=128, RoPE, causal) on 8 trn2 NeuronCores.

Sharding: mesh = 2 (batch) x 4 (KV-head groups).  Core c = b*4 + g gets
batch b and KV head g together with its 4 query heads (tensor parallel on
the head dim: q/k/v projection output dim and o-proj input dim).  Each core
produces a partial y[b] (o-proj over its 512 input dims); host sums the 4
partials per batch.

On-chip layout: all activations transposed ([feature, seq]) so every matmul
contracts along the partition dim.  DMA'd tensors (x, weights, y) travel as
bf16 (half the HBM traffic, same 1-cycle/row PE rate); on-chip
intermediates (q/k/v, exp scores) stay float32r.  Softmax is unnormalized:
exp(scale*s) via ACT, denominator via an all-ones [128,128] stationary
matmul that lands pre-broadcast in PSUM, reciprocal on the ACT engine,
causal mask as a post-exp 0/1 multiply on DVE.
"""

import os

import numpy as np

S = 2048
D = 2048
HD = 128
NQH = 16
NKVH = 4
GROUPS = NQH // NKVH  # 4 q heads per kv head
O = GROUPS * HD  # 512 per-core q/o slice
NB = 2
NCORES = 8
SCALE = 1.0 / float(np.sqrt(np.float32(HD)))

SBLK = 512  # seq block for projections / sq block in attention
NKB = S // HD  # 16 128-blocks along seq
NSB = S // SBLK  # 4 512-blocks along seq
NDB = D // HD  # 16 d blocks

LAST_EXEC_NS = None
LAST_TRACE = None

_CACHE = {}


def _rope_tables():
    k = np.arange(0, HD, 2)[: HD // 2].astype(np.float32)
    inv_freq = (1.0 / 10000.0 ** (k / HD)).astype(np.float32)
    positions = np.arange(S, dtype=np.float32)
    ang = positions[:, None] * inv_freq[None, :]  # [S, 64]
    ang = np.concatenate([ang, ang], axis=-1)  # [S, 128]
    cosT = np.cos(ang).astype(np.float32).T  # [128, S]
    sinT = np.sin(ang).astype(np.float32).T
    return np.ascontiguousarray(cosT), np.ascontiguousarray(sinT)


def _mask_table():
    # maskM[i, j*512 + s] = 1 if (j*128 + i) <= s else 0  (keep-mask)
    m = np.empty((HD, 4 * SBLK), dtype=np.float32)
    i = np.arange(HD)[:, None]
    s = np.arange(SBLK)[None, :]
    for j in range(4):
        m[:, j * SBLK : (j + 1) * SBLK] = np.where(j * HD + i <= s, 1.0, 0.0)
    return m


def _shift_table():
    # rot = P @ q  with rot[i] = -q[i+64] (i<64), q[i-64] (i>=64); ship P.T
    P = np.zeros((HD, HD), dtype=np.float32)
    h = HD // 2
    P[np.arange(h), np.arange(h) + h] = -1.0
    P[np.arange(h) + h, np.arange(h)] = 1.0
    return np.ascontiguousarray(P.T)


def _build_program():
    import concourse.bass as bass
    import concourse.mybir as mybir
    from concourse.tile import TileContext

    f32 = mybir.dt.float32
    f32r = mybir.dt.float32r
    bf16 = mybir.dt.bfloat16
    EXP = mybir.ActivationFunctionType.Exp
    LN = mybir.ActivationFunctionType.Ln

    nc = bass.Bass()

    xT = nc.declare_dram_parameter("xT", [D, S], bf16, isOutput=False)
    wqP = nc.declare_dram_parameter("wqP", [128, NDB * O], bf16, isOutput=False)
    wkP = nc.declare_dram_parameter("wkP", [128, NDB * HD], bf16, isOutput=False)
    wvP = nc.declare_dram_parameter("wvP", [128, NDB * HD], bf16, isOutput=False)
    woP = nc.declare_dram_parameter("woP", [128, GROUPS * D], bf16, isOutput=False)
    cosT = nc.declare_dram_parameter("cosT", [HD, S], bf16, isOutput=False)
    sinT = nc.declare_dram_parameter("sinT", [HD, S], bf16, isOutput=False)
    maskM = nc.declare_dram_parameter("maskM", [HD, 4 * SBLK], bf16, isOutput=False)
    shiftPT = nc.declare_dram_parameter("shiftPT", [HD, HD], f32r, isOutput=False)
    ident = nc.declare_dram_parameter("ident", [HD, HD], f32r, isOutput=False)
    onesmat = nc.declare_dram_parameter("onesmat", [HD, HD], f32r, isOutput=False)
    y = nc.declare_dram_parameter("y", [S, D], bf16, isOutput=True)

    with TileContext(nc) as tc:
        with tc.tile_pool(name="persist", bufs=1) as pp:
            wq_sb = pp.tile([128, NDB * O], bf16, name="wq_sb")  # [d_blk][128d, 512o]
            wk_sb = pp.tile([128, NDB * HD], bf16, name="wk_sb")
            wv_sb = pp.tile([128, NDB * HD], bf16, name="wv_sb")
            wo_sb = pp.tile([128, GROUPS * D], bf16, name="wo_sb")  # [o_blk][128o, 2048]
            cos_sb = pp.tile([128, S], bf16, name="cos_sb")
            sin_sb = pp.tile([128, S], bf16, name="sin_sb")
            mask_sb = pp.tile([128, 4 * SBLK], bf16, name="mask_sb")
            shift_sb = pp.tile([128, HD], f32r, name="shift_sb")
            id_sb = pp.tile([128, HD], f32r, name="id_sb")
            ones_sb = pp.tile([128, HD], f32r, name="ones_sb")
            q_sb = pp.tile([128, GROUPS * S], f32r, name="q_sb")  # per head [128hd, S]
            k_sb = pp.tile([128, S], f32r, name="k_sb")
            v_sb = pp.tile([128, NKB * HD], f32r, name="v_sb")  # [s_blk][128s, 128hd]

            # weight loads in 4-db chunks on the scalar (ACT) DMA queue
            # (nothing else rides scalar, so phase-1 ACT copies are never
            # stuck behind DMA issue depth-waits); small tables + sb0's
            # cos/sin chunk lead the sync queue, x tiles follow
            CH = 4
            for c in range(NDB // CH):
                nc.scalar.dma_start(
                    out=wq_sb[:, c * CH * O : (c + 1) * CH * O],
                    in_=wqP[:, c * CH * O : (c + 1) * CH * O],
                )
                nc.scalar.dma_start(
                    out=wk_sb[:, c * CH * HD : (c + 1) * CH * HD],
                    in_=wkP[:, c * CH * HD : (c + 1) * CH * HD],
                )
            nc.sync.dma_start(out=shift_sb[:], in_=shiftPT[:])
            nc.sync.dma_start(out=id_sb[:], in_=ident[:])
            nc.sync.dma_start(out=ones_sb[:], in_=onesmat[:])
            nc.sync.dma_start(out=cos_sb[:, :SBLK], in_=cosT[:, :SBLK])
            nc.sync.dma_start(out=sin_sb[:, :SBLK], in_=sinT[:, :SBLK])

            # ---------------- Phase 1: projections + RoPE + v transpose
            with (
                tc.tile_pool(name="p1acc", bufs=6, space="PSUM") as accp,
                tc.tile_pool(name="p1rot", bufs=2, space="PSUM") as rotp,
                tc.tile_pool(name="xts", bufs=4) as xpool,
                tc.tile_pool(name="raws", bufs=8) as rawpool,
                tc.tile_pool(name="tmps", bufs=4) as tmppool,
            ):
                for sb in range(NSB):
                    sl = slice(sb * SBLK, (sb + 1) * SBLK)
                    ps = [
                        accp.tile([128, SBLK], f32, name=f"acc{i}_{sb}", tag="acc")
                        for i in range(6)
                    ]  # q0..q3, k, v
                    for db in range(NDB):
                        if sb == 0 and db % CH == 0:
                            c = db // CH
                            nc.sync.dma_start(
                                out=wv_sb[:, c * CH * HD : (c + 1) * CH * HD],
                                in_=wvP[:, c * CH * HD : (c + 1) * CH * HD],
                            )
                        xt = xpool.tile([128, SBLK], bf16, name=f"xt{sb}_{db}", tag="xt")
                        nc.sync.dma_start(
                            out=xt[:], in_=xT[db * 128 : (db + 1) * 128, sl]
                        )
                        st = db == 0
                        sp = db == NDB - 1
                        for ob in range(GROUPS):
                            nc.tensor.matmul(
                                ps[ob][:],
                                wq_sb[:, db * O + ob * 128 : db * O + (ob + 1) * 128],
                                xt[:],
                                start=st,
                                stop=sp,
                            )
                        nc.tensor.matmul(
                            ps[4][:],
                            wk_sb[:, db * HD : (db + 1) * HD],
                            xt[:],
                            start=st,
                            stop=sp,
                        )
                        nc.tensor.matmul(
                            ps[5][:],
                            wv_sb[:, db * HD : (db + 1) * HD],
                            xt[:],
                            start=st,
                            stop=sp,
                        )
                    if sb < NSB - 1:
                        nsl = slice((sb + 1) * SBLK, (sb + 2) * SBLK)
                        nc.sync.dma_start(out=cos_sb[:, nsl], in_=cosT[:, nsl])
                        nc.sync.dma_start(out=sin_sb[:, nsl], in_=sinT[:, nsl])
                    # stage all six psum accumulators to SBUF first: the
                    # accp banks free after ~4us of ACT copies, so the next
                    # sb's projections never wait on the rope/transpose chain
                    raws = []
                    for i in range(5):
                        raw = rawpool.tile([128, SBLK], f32r, name=f"raw{sb}_{i}", tag="raw")
                        nc.scalar.copy(raw[:], ps[i][:])
                        raws.append(raw)
                    vst = rawpool.tile([128, SBLK], f32r, name=f"vst{sb}", tag="raw")
                    nc.scalar.copy(vst[:], ps[5][:])
                    # RoPE on q heads and k
                    for i in range(5):
                        dst = (
                            q_sb[:, i * S + sb * SBLK : i * S + (sb + 1) * SBLK]
                            if i < 4
                            else k_sb[:, sl]
                        )
                        raw = raws[i]
                        rot = rotp.tile([128, SBLK], f32, name=f"rot{sb}_{i}", tag="rot")
                        nc.tensor.matmul(
                            rot[:], shift_sb[:], raw[:], start=True, stop=True
                        )
                        tmp = tmppool.tile([128, SBLK], f32, name=f"tmp{sb}_{i}", tag="tmp")
                        nc.vector.tensor_mul(tmp[:], raw[:], cos_sb[:, sl])
                        t2 = tmppool.tile([128, SBLK], f32, name=f"t2_{sb}_{i}", tag="tmp")
                        nc.vector.tensor_mul(t2[:], rot[:], sin_sb[:, sl])
                        nc.vector.tensor_add(dst, tmp[:], t2[:])
                    # v: PE-transpose 128x128 blocks to natural layout
                    for sub in range(SBLK // HD):
                        vt = rotp.tile([128, SBLK], f32r, name=f"vt{sb}_{sub}", tag="rot")
                        nc.tensor.transpose(
                            vt[:, :HD], vst[:, sub * HD : (sub + 1) * HD], id_sb[:]
                        )
                        kb = sb * 4 + sub
                        nc.scalar.copy(v_sb[:, kb * HD : (kb + 1) * HD], vt[:, :HD])

            nc.sync.dma_start(out=mask_sb[:], in_=maskM[:])
            nc.sync.dma_start(out=wo_sb[:], in_=woP[:])

            # ---------------- Phase 2: attention + o-proj, per sq block.
            # kb blocks processed in pairs: sc/e tiles are [128, 1024] so
            # the ACT exp and DVE mask amortize their fixed access latency.
            with (
                tc.tile_pool(name="p2sc", bufs=2, space="PSUM") as scp,
                tc.tile_pool(name="p2av", bufs=2, space="PSUM") as avp,
                tc.tile_pool(name="p2den", bufs=2, space="PSUM") as denp,
                tc.tile_pool(name="exps", bufs=6) as epool,
                tc.tile_pool(name="recs", bufs=4) as recpool,
                tc.tile_pool(name="aos", bufs=8) as aopool,
                tc.tile_pool(name="ysb", bufs=3) as ypool_sb,
            ):
                pending_oproj = [None]

                for sq in range(NSB):
                    nsk = 4 * sq + 4
                    aoh = [
                        aopool.tile([128, SBLK], bf16, name=f"ao{sq}_{h}", tag="ao")
                        for h in range(GROUPS)
                    ]
                    # flattened (head, pair) stream with 1-pair sc prefetch:
                    # the sc matmuls of pair i+1 are issued while pair i's
                    # exp is still on the ACT engine, so the PE never waits
                    # a full exp latency at head boundaries.  Diagonal
                    # (masked) pairs go first within each head so the DVE
                    # mask-mul hides behind the long unmasked run.
                    kps = [2 * sq, 2 * sq + 1] + list(range(2 * sq))
                    stream = [(h, ki, kp) for h in range(GROUPS) for ki, kp in enumerate(kps)]
                    np2 = len(kps)

                    sc_t = {}
                    avd = {}

                    def emit_sc(idx):
                        h, ki, kp = stream[idx]
                        if ki == 0:
                            avd[h] = (
                                avp.tile([128, SBLK], f32, name=f"av{sq}_{h}", tag="av"),
                                denp.tile([128, SBLK], f32, name=f"den{sq}_{h}", tag="den"),
                            )
                        qsl = q_sb[:, h * S + sq * SBLK : h * S + (sq + 1) * SBLK]
                        sc = scp.tile([128, 2 * SBLK], f32, name=f"sc{sq}_{h}_{kp}", tag="sc")
                        for half, kb in ((0, 2 * kp), (1, 2 * kp + 1)):
                            nc.tensor.matmul(
                                sc[:, half * SBLK : (half + 1) * SBLK],
                                k_sb[:, kb * HD : (kb + 1) * HD],
                                qsl,
                                start=True,
                                stop=True,
                            )
                        e = epool.tile([128, 2 * SBLK], f32r, name=f"e{sq}_{h}_{kp}", tag="e")
                        nc.scalar.activation(e[:], sc[:], EXP, scale=SCALE)
                        if kp >= 2 * sq:
                            j2 = kp - 2 * sq
                            nc.vector.tensor_mul(
                                e[:], e[:], mask_sb[:, j2 * 2 * SBLK : (j2 + 1) * 2 * SBLK]
                            )
                        sc_t[idx] = e

                    pending_recip = [None]
                    emit_sc(0)
                    for idx in range(len(stream)):
                        h, ki, kp = stream[idx]
                        if idx + 1 < len(stream):
                            emit_sc(idx + 1)
                        if idx == 2 and pending_oproj[0] is not None:
                            # previous sq's o-proj runs here: its last-head
                            # reciprocal chain hides behind this sq's first
                            # score pairs
                            pending_oproj[0]()
                            pending_oproj[0] = None
                        e = sc_t.pop(idx)
                        av, den = avd[h]
                        for half, kb in ((0, 2 * kp), (1, 2 * kp + 1)):
                            esl = e[:, half * SBLK : (half + 1) * SBLK]
                            st = ki == 0 and half == 0
                            sp = ki == np2 - 1 and half == 1
                            nc.tensor.matmul(
                                av[:],
                                v_sb[:, kb * HD : (kb + 1) * HD],
                                esl,
                                start=st,
                                stop=sp,
                            )
                            nc.tensor.matmul(den[:], ones_sb[:], esl, start=st, stop=sp)
                        if pending_recip[0] is not None:
                            # previous head's 1/den, delayed one pair so the
                            # ACT queue clears this head's first exps before
                            # the 1.3us ln+exp insertion
                            pending_recip[0]()
                            pending_recip[0] = None
                        if ki == np2 - 1:
                            def _mk_recip(h, av, den):
                                def _go():
                                    # 1/den as exp(-ln(den)) on ACT (DVE-free;
                                    # ACT Reciprocal is gated in bass)
                                    lnt = recpool.tile([128, SBLK], f32, name=f"ln{sq}_{h}", tag="rec")
                                    nc.scalar.activation(lnt[:], den[:], LN)
                                    rec = recpool.tile([128, SBLK], f32, name=f"rec{sq}_{h}", tag="rec")
                                    nc.scalar.activation(rec[:], lnt[:], EXP, scale=-1.0)
                                    nc.vector.tensor_mul(aoh[h][:], av[:], rec[:])
                                return _go
                            pending_recip[0] = _mk_recip(h, av, den)
                    if pending_recip[0] is not None:
                        pending_recip[0]()
                        pending_recip[0] = None
                    # o-proj for this sq block, deferred into the next
                    # sq's pair stream (final sq drains at the very end,
                    # alternating ysb copies across ACT/DVE)
                    def _mk_oproj(sq, aoh):
                        def _go(final=False):
                            for sub in range(SBLK // HD):
                                for dcp in range(D // (2 * SBLK)):
                                    yt = scp.tile(
                                        [128, 2 * SBLK], f32, name=f"y{sq}_{sub}_{dcp}", tag="sc"
                                    )
                                    for half in range(2):
                                        dc = 2 * dcp + half
                                        for ob in range(GROUPS):
                                            nc.tensor.matmul(
                                                yt[:, half * SBLK : (half + 1) * SBLK],
                                                aoh[ob][:, sub * HD : (sub + 1) * HD],
                                                wo_sb[:, ob * D + dc * SBLK : ob * D + (dc + 1) * SBLK],
                                                start=(ob == 0),
                                                stop=(ob == GROUPS - 1),
                                            )
                                    ysb = ypool_sb.tile(
                                        [128, 2 * SBLK], bf16, name=f"ysb{sq}_{sub}_{dcp}", tag="ysb"
                                    )
                                    if final and (sub + dcp) % 2 == 0:
                                        nc.scalar.copy(ysb[:], yt[:])
                                    else:
                                        nc.vector.tensor_copy(ysb[:], yt[:])
                                    nc.sync.dma_start(
                                        out=y[
                                            sq * SBLK + sub * HD : sq * SBLK + (sub + 1) * HD,
                                            dcp * 2 * SBLK : (dcp + 1) * 2 * SBLK,
                                        ],
                                        in_=ysb[:],
                                    )
                        return _go
                    pending_oproj[0] = _mk_oproj(sq, aoh)
                if pending_oproj[0] is not None:
                    pending_oproj[0](final=True)
    _split_matmul_waits(nc, mybir)
    return nc


def _split_matmul_waits(nc, mybir):
    """TRN2 instructions can carry only one HW sync-wait command; Tile
    sometimes attaches several.  Move the extras onto nofuse nops on the
    same engine inserted just before the instruction."""
    for f in nc.m.functions:
        for bb in f.blocks:
            insts = bb.instructions
            fixes = []
            for idx, inst in enumerate(insts):
                si = inst.sync_info
                if si is None or len(si.on_wait) <= 1:
                    continue
                fixes.append((idx, inst, list(si.on_wait), list(si.on_update)))
            for idx, inst, waits, updates in reversed(fixes):
                inst.sync_info = mybir.SyncInfo(on_wait=[waits[-1]], on_update=updates)
                for w in reversed(waits[:-1]):
                    nop = mybir.InstNoOp(
                        name=nc.get_next_instruction_name(), ins=[], outs=[]
                    )
                    nop.engine = inst.engine
                    nop.bass_nofuse = True
                    nop.sync_info = mybir.SyncInfo(on_wait=[w], on_update=[])
                    insts.insert(idx, nop)


def _per_core_inputs(x, Wq, Wk, Wv, Wo):
    import ml_dtypes

    bf16 = ml_dtypes.bfloat16
    cosT, sinT = _rope_tables()
    maskM = _mask_table()
    shiftPT = _shift_table()
    ident = np.eye(HD, dtype=np.float32)
    onesmat = np.ones((HD, HD), dtype=np.float32)
    in_maps = []
    for b in range(NB):
        xTb = np.ascontiguousarray(x[b].T.astype(bf16))
        for g in range(NKVH):
            wqT = Wq[g * O : (g + 1) * O, :].T  # [D, O]
            wkT = Wk[g * HD : (g + 1) * HD, :].T
            wvT = Wv[g * HD : (g + 1) * HD, :].T
            woT = Wo[:, g * O : (g + 1) * O].T  # [O, D]
            in_maps.append(
                {
                    "xT": xTb,
                    "wqP": np.ascontiguousarray(
                        wqT.reshape(NDB, 128, O).transpose(1, 0, 2).reshape(128, NDB * O)
                    ).astype(bf16),
                    "wkP": np.ascontiguousarray(
                        wkT.reshape(NDB, 128, HD).transpose(1, 0, 2).reshape(128, NDB * HD)
                    ).astype(bf16),
                    "wvP": np.ascontiguousarray(
                        wvT.reshape(NDB, 128, HD).transpose(1, 0, 2).reshape(128, NDB * HD)
                    ).astype(bf16),
                    "woP": np.ascontiguousarray(
                        woT.reshape(GROUPS, 128, D).transpose(1, 0, 2).reshape(128, GROUPS * D)
                    ).astype(bf16),
                    "cosT": cosT.astype(bf16),
                    "sinT": sinT.astype(bf16),
                    "maskM": maskM.astype(bf16),
                    "shiftPT": shiftPT,
                    "ident": ident,
                    "onesmat": onesmat,
                }
            )
    return in_maps


def kernel(x, Wq, Wk, Wv, Wo):
    global LAST_EXEC_NS, LAST_TRACE
    from concourse.bass_utils import run_bass_kernel_spmd

    if "nc" not in _CACHE:
        _CACHE["nc"] = _build_program()
    nc = _CACHE["nc"]

    x = np.asarray(x)
    in_maps = _per_core_inputs(
        x, np.asarray(Wq), np.asarray(Wk), np.asarray(Wv), np.asarray(Wo)
    )
    trace = bool(os.environ.get("KERNEL_PROFILE"))
    res = run_bass_kernel_spmd(
        nc, in_maps, core_ids=list(range(NCORES)), trace=trace
    )
    globals()["LAST_RESULT"] = res
    LAST_EXEC_NS = res.exec_time_ns
    LAST_TRACE = getattr(res, "profile_json", None)
    out = np.empty((NB, S, D), dtype=np.float32)
    for b in range(NB):
        acc = res.results[b * NKVH]["y"].astype(np.float32)
        for g in range(1, NKVH):
            acc += res.results[b * NKVH + g]["y"].astype(np.float32)
        out[b] = acc
    return out
